# revision 16
# baseline (speedup 1.0000x reference)
"""Trainium2 Bass kernel for nn_MultiHeadRelativeAttention (S=256, E=1024, H=16).

Sharding: tensor-parallel over heads. Each of 8 cores owns 2 heads (128 dims),
computes its slice of projections/scores/softmax/attn, and emits a full-shape
(256, 1024) partial of the output projection; the host sums the 8 partials.

Device decomposition per core (all bf16 on the PE):
  - a2 = nq_i . s_k[i]: 2-packed. Moving column j of pair p=(i1,i2)=(64g+p,
    64g+p+128) holds [sk(i1,j,d) d<64 ; sk(i2,j,d)] across all 128 partitions
    (host pre-packs). The stationary is a 128-col window into a zero-spaced
    buffer B2 whose col 128p rows 0:64 = nq(i1), col 128p+64 rows 64:128 =
    nq(i2); window [127p, 127p+128) exposes exactly those two at relative
    cols p and 64+p, so PSUM row p accumulates a2(i1,:) and row 64+p a2(i2,:).
    64 pairs accumulate into one (128,256) PSUM tile per (g,h) group; pair 0
    opens the accumulation group, a1 and the a3 identity-add close it.
  - a1 = nq . key: block-diagonal stationary (nq(i1-set) top-left 64x64,
    nq(i2-set) bottom-right) x key duplicated across both partition halves.
  - a3: rel-shift via DRAM bounce (store rows at stride 512 into a zero-padded
    scratch, read back with an affine AP), identity-matmul'd into the group.
  - mask folded as multiply by (1-mask)*scaling (exp(0)==exp(1e-30)==1.0).
  - softmax without max-subtraction; denominators applied to attn@v rows.

Schedule: the 16.8 MB s_k stream is split across the sync ring (even 1 MB
chunks) and vector ring (odd); consts flow on the scalar ring in need-order;
the a3 bounce and outputs use the gpsimd ring. B2 is double-buffered so the
next group's stationary scatter overlaps the current group's matmuls, keeping
the PE p-state at full clock. keyT/relT/value projections fill PE slots
between early chunks.
"""

import sys

if "/opt/trn_rl_repo" not in sys.path:
    sys.path.insert(0, "/opt/trn_rl_repo")

import numpy as np

import concourse.bass as bass
import concourse.mybir as mybir
import concourse.tile as tile
from concourse import bacc
from concourse.masks import make_identity

S = 256
E = 1024
H = 16
HD = 64
NCORES = 8
DHB = 128          # head-dim block per core (2 heads x 64)
SCALING = float(HD) ** -0.5

F32 = mybir.dt.float32
BF16 = mybir.dt.bfloat16
NPBF = np.dtype("bfloat16")

NCHUNK = 16        # s_k chunks (1 MB: 16 pairs x 256 j x 128 part bf16)
PPC = 16           # pairs per chunk


def emit(tc: tile.TileContext, t: dict):
    nc = tc.nc
    from contextlib import ExitStack

    ctx = ExitStack()
    const = ctx.enter_context(tc.tile_pool(name="const", bufs=1))
    skp = ctx.enter_context(tc.tile_pool(name="skp", bufs=10))
    work = ctx.enter_context(tc.tile_pool(name="work", bufs=2))
    psS = ctx.enter_context(tc.tile_pool(name="psS", bufs=2, space="PSUM"))
    psM = ctx.enter_context(tc.tile_pool(name="psM", bufs=2, space="PSUM"))

    hs = [slice(0, 64), slice(64, 128)]

    # ---- const tiles ----
    xq = const.tile([128, 8, 256], BF16, tag="xq")
    xk = const.tile([128, 8, 256], BF16, tag="xk")
    xp = const.tile([128, 8, 256], BF16, tag="xp")
    xv = const.tile([128, 8, 256], BF16, tag="xv")
    wq = const.tile([128, 8, 128], BF16, tag="wq")
    wk = const.tile([128, 8, 128], BF16, tag="wk")
    wr = const.tile([128, 8, 128], BF16, tag="wr")
    wv = const.tile([128, 8, 128], BF16, tag="wv")
    wo = const.tile([128, 1024], BF16, tag="wo")
    sq = const.tile([128, 1], F32, tag="sq")
    mnot = const.tile([128, 4, 256], BF16, tag="mnot")

    # skT chunks alternate rings (even: sync, odd: scalar) so the early
    # stream is fed at two-ring bandwidth.
    skt_q = {}

    def load_chunk(idx):
        skc = skp.tile([128, PPC, 256], BF16, tag="skc", name=f"skc{idx}")
        eng = nc.sync
        eng.dma_start(out=skc, in_=t["skT"][:, 4096 * idx:4096 * (idx + 1)])
        skt_q[idx] = skc

    # critical consts lead their rings; bulk consts ride the gpsimd ring
    nc.scalar.dma_start(out=wq, in_=t["wq"])
    nc.scalar.dma_start(out=xq, in_=t["xq"])
    nc.scalar.dma_start(out=sq, in_=t["sq"])
    nc.scalar.dma_start(out=xp, in_=t["xp"])
    nc.scalar.dma_start(out=wr, in_=t["wr"])
    nc.scalar.dma_start(out=xk, in_=t["xk"])
    nc.scalar.dma_start(out=wk, in_=t["wk"])
    nc.scalar.dma_start(out=xv, in_=t["xv"])
    nc.scalar.dma_start(out=wv, in_=t["wv"])
    nc.scalar.dma_start(out=mnot, in_=t["mnot"])
    nc.scalar.dma_start(out=wo, in_=t["wo"])
    for idx in range(6):
        load_chunk(idx)

    # ---- zero-fills + gpsimd-ring consts ----
    B2 = [const.tile([128, 8192], BF16, tag=f"B2{i}", name=f"B2{i}")
          for i in range(2)]
    A1z = [const.tile([128, 128], BF16, tag=f"A1z{i}", name=f"A1z{i}")
           for i in range(2)]
    zt = const.tile([128, 512], BF16, tag="zt")
    nc.vector.memset(B2[0][:, 0:4096], 0.0)
    nc.vector.memset(zt, 0.0)
    nc.gpsimd.memset(B2[0][:, 4096:8192], 0.0)
    nc.gpsimd.memset(B2[1][:, 0:4096], 0.0)
    nc.gpsimd.memset(A1z[0], 0.0)
    nc.gpsimd.memset(A1z[1], 0.0)
    identb = const.tile([128, 128], BF16, tag="identb")
    make_identity(nc, identb)
    for h in range(2):
        scr = t[f"a3scr{h}"]
        nc.scalar.dma_start(
            out=bass.AP(tensor=scr.tensor, offset=scr.offset + 256,
                        ap=[[512, 128], [65536, 2], [1, 256]]),
            in_=zt)

    # ---- projections: (128 dh, 256 s), accumulated over 8 E-chunks ----
    def proj_T(wsb, xsb):
        ps = psM.tile([128, 256], F32, tag="pm", name="ps_proj")
        for c in range(8):
            nc.tensor.matmul(ps, wsb[:, c, :], xsb[:, c, :],
                             start=(c == 0), stop=(c == 7))
        return ps

    nqT = const.tile([128, 256], BF16, tag="nqT")
    nc.vector.tensor_scalar_add(out=nqT, in0=proj_T(wq, xq), scalar1=sq)

    # ---- PE filler work, slotted between group-0 chunks ----
    keyT = const.tile([128, 256], BF16, tag="keyT")
    relT = const.tile([128, 256], BF16, tag="relT")
    value = [const.tile([128, 128], BF16, tag=f"value{jh}", name=f"value{jh}")
             for jh in range(2)]
    keyd = [const.tile([128, 256], BF16, tag=f"keyd{h}", name=f"keyd{h}")
            for h in range(2)]

    def fill_relT_raws():
        nc.vector.tensor_copy(out=relT, in_=proj_T(wr, xp))
        for h in range(2):
            raw = work.tile([128, 2, 256], BF16, tag="a3raw")
            for ib in range(2):
                ps = psM.tile([128, 256], F32, tag="pm", name=f"ps_a3_{h}{ib}")
                nc.tensor.matmul(ps, nqT[hs[h], 128 * ib:128 * ib + 128],
                                 relT[hs[h], :], start=True, stop=True)
                nc.scalar.copy(out=raw[:, ib, :], in_=ps)
            scr = t[f"a3scr{h}"]
            nc.scalar.dma_start(
                out=bass.AP(tensor=scr.tensor, offset=scr.offset,
                            ap=[[512, 128], [65536, 2], [1, 256]]),
                in_=raw)

    def fill_keyT():
        nc.vector.tensor_copy(out=keyT, in_=proj_T(wk, xk))
        for h in range(2):
            nc.vector.tensor_copy(out=keyd[h][0:64, :], in_=keyT[hs[h], :])
            nc.vector.tensor_copy(out=keyd[h][64:128, :], in_=keyT[hs[h], :])

    def fill_value():
        for jh in range(2):
            ps = psM.tile([128, 128], F32, tag="pm", name=f"ps_val{jh}")
            for c in range(8):
                nc.tensor.matmul(ps, xv[:, c, 128 * jh:128 * jh + 128],
                                 wv[:, c, :], start=(c == 0), stop=(c == 7))
            nc.vector.tensor_copy(out=value[jh], in_=ps)

    # ---- score groups: gi = 2*g + h ----
    def scatter_group(gi):
        """stationary scatter for group gi into B2[gi%2] / A1z[:,gi%2,:]."""
        g, h = divmod(gi, 2)
        Bv = B2[gi % 2].rearrange("p (m q) -> p m q", q=128)
        nc.vector.tensor_copy(out=Bv[0:64, :, 0],
                              in_=nqT[hs[h], 64 * g:64 * g + 64])
        nc.vector.tensor_copy(out=Bv[64:128, :, 64],
                              in_=nqT[hs[h], 64 * g + 128:64 * g + 192])
        a1st = A1z[gi % 2]
        nc.vector.tensor_copy(out=a1st[0:64, 0:64],
                              in_=nqT[hs[h], 64 * g:64 * g + 64])
        nc.vector.tensor_copy(out=a1st[64:128, 64:128],
                              in_=nqT[hs[h], 64 * g + 128:64 * g + 192])

    def read_a3s(gi):
        g, h = divmod(gi, 2)
        a3s = work.tile([128, 256], BF16, tag="a3s", name=f"a3s{gi}")
        scr = t[f"a3scr{h}"]
        nc.scalar.dma_start(
            out=a3s,
            in_=bass.AP(tensor=scr.tensor,
                        offset=scr.offset + 255 + 511 * 64 * g,
                        ap=[[65536 - 128, 2], [511, 64], [1, 256]]))
        return a3s

    def consume_chunk(Sp, gi, ch):
        idx = 4 * gi + ch
        skc = skt_q.pop(idx)
        if idx + 6 < NCHUNK:
            load_chunk(idx + 6)
        B = B2[gi % 2]
        for pp in range(PPC):
            p = PPC * ch + pp
            nc.tensor.matmul(Sp, B[:, 127 * p:127 * p + 128], skc[:, pp, :],
                             start=(p == 0), stop=False)

    def close_group(Sp, a3s, gi):
        h = gi % 2
        nc.tensor.matmul(Sp, A1z[gi % 2], keyd[h], start=False,
                         stop=False)
        nc.tensor.matmul(Sp, identb, a3s, start=False, stop=True)

    attn_g = [const.tile([128, 128], BF16, tag=f"attn{g}", name=f"attn{g}")
              for g in range(2)]
    rden_q = {}

    def tail_soft(Sp, gi):
        """softmax numerators (vector/scalar engines only)."""
        w2 = work.tile([128, 256], F32, tag="w2")
        nc.vector.tensor_mul(out=w2, in0=Sp, in1=mnot[:, gi, :])
        ex = work.tile([128, 256], BF16, tag="ex", name=f"ex{gi}")
        nc.scalar.activation(out=ex, in_=w2,
                             func=mybir.ActivationFunctionType.Exp, scale=1.0)
        den = work.tile([128, 1], F32, tag="den")
        nc.vector.reduce_sum(out=den, in_=ex, axis=mybir.AxisListType.X)
        rden = work.tile([128, 1], F32, tag="rden", name=f"rden{gi}")
        nc.vector.reciprocal(out=rden, in_=den)
        rden_q[gi] = (ex, rden)

    def tail_pe(gi):
        """attn@v for one group (PE + scalar + vector)."""
        g, h = divmod(gi, 2)
        ex, rden = rden_q.pop(gi)
        av = psM.tile([128, 64], F32, tag="pm", name=f"av{gi}")
        for jh in range(2):
            tp = psM.tile([128, 128], BF16, tag="tp", name=f"tp{gi}{jh}")
            nc.tensor.transpose(tp, ex[:, 128 * jh:128 * jh + 128], identb)
            st = work.tile([128, 128], BF16, tag="st")
            nc.vector.tensor_copy(out=st, in_=tp)
            nc.tensor.matmul(av, st, value[jh][:, hs[h]],
                             start=(jh == 0), stop=(jh == 1))
        nc.vector.tensor_scalar_mul(out=attn_g[g][:, hs[h]], in0=av,
                                    scalar1=rden)

    def outproj(g):
        tpa = psM.tile([128, 128], BF16, tag="tp", name=f"tpa{g}")
        nc.tensor.transpose(tpa, attn_g[g], identb)
        aT = work.tile([128, 128], BF16, tag="aT")
        nc.vector.tensor_copy(out=aT, in_=tpa)
        out_sb = const.tile([128, 1024], BF16, tag=f"out{g}", name=f"out{g}")
        for nh in range(2):
            op = psM.tile([128, 512], F32, tag="op", name=f"op{g}{nh}")
            nc.tensor.matmul(op, aT, wo[:, 512 * nh:512 * (nh + 1)],
                             start=True, stop=True)
            cols = slice(512 * nh, 512 * (nh + 1))
            nc.vector.tensor_copy(out=out_sb[:, cols], in_=op)
            outp = t["outp"]
            nc.scalar.dma_start(
                out=bass.AP(tensor=outp.tensor,
                            offset=outp.offset + 64 * g * 1024 + 512 * nh,
                            ap=[[131072, 2], [1024, 64], [1, 512]]),
                in_=out_sb[:, cols])

    # ---- main schedule ----
    # group-0 fillers keep the PE fed while consts trickle in; each group's
    # close (a1 + a3-identity) is emitted after the NEXT group's first chunk
    # so a3 readback latency never stalls the a2 stream.
    fillers = {(0, 0): fill_relT_raws, (0, 1): fill_keyT, (1, 1): fill_value}

    Sps = {}
    a3q = {}
    for gi in range(4):
        scatter_group(gi)
        if gi == 0:
            nc.vector.memset(B2[1][:, 4096:8192], 0.0)
        Sps[gi] = psS.tile([128, 256], F32, tag="S", name=f"S{gi}")
        for ch in range(4):
            consume_chunk(Sps[gi], gi, ch)
            f = fillers.pop((gi, ch), None)
            if f:
                f()
            if ch == 0 and gi >= 1:
                close_group(Sps[gi - 1], a3q[gi - 1], gi - 1)
                tail_soft(Sps[gi - 1], gi - 1)
            if ch == 1:
                a3q[gi] = read_a3s(gi)
            if ch == 2 and gi >= 1:
                tail_pe(gi - 1)
            if ch == 2 and gi == 2:
                outproj(0)
    close_group(Sps[3], a3q[3], 3)
    tail_soft(Sps[3], 3)
    tail_pe(3)
    outproj(1)

    ctx.close()


def build():
    nc = bacc.Bacc("TRN2", target_bir_lowering=False, debug=False)
    t = {}

    def inp(name, shape, dt=BF16):
        t[name] = nc.dram_tensor(name, list(shape), dt, kind="ExternalInput").ap()

    inp("skT", (128, S * S))
    for n in ("xq", "xk", "xp", "xv"):
        inp(n, (128, 8 * 256))
    for n in ("wq", "wk", "wr", "wv"):
        inp(n, (128, 8 * 128))
    inp("wo", (DHB, E))
    inp("sq", (DHB, 1), F32)
    inp("mnot", (128, 4 * 256))
    for h in range(2):
        t[f"a3scr{h}"] = nc.dram_tensor(f"a3scr{h}", [256, 512], BF16).ap()
    t["outp"] = nc.dram_tensor("outp", [S, E], BF16,
                               kind="ExternalOutput").ap()

    with tile.TileContext(nc) as tc:
        emit(tc, t)
    nc.compile()
    return nc


def make_in_maps(inputs: dict) -> list[dict]:
    q = np.asarray(inputs["q"], np.float32)
    k = np.asarray(inputs["k"], np.float32)
    v = np.asarray(inputs["v"], np.float32)
    p = np.asarray(inputs["p"], np.float32)
    mask = np.asarray(inputs["mask"])
    s_q = np.asarray(inputs["s_q"], np.float32)
    s_k = np.asarray(inputs["s_k"], np.float32)
    W = {n: np.asarray(inputs[n], np.float32)
         for n in ("Wq", "Wk", "Wv", "Wr", "Wo")}

    def xarr(x):
        # (S, E) -> (128 p, 8 c, 256 s) where E-index = 128*c + p
        return np.ascontiguousarray(
            x.T.reshape(8, 128, S).transpose(1, 0, 2)).astype(NPBF).reshape(128, -1)

    xs = {"xq": xarr(q), "xk": xarr(k), "xp": xarr(p), "xv": xarr(v)}

    maps = []
    for c in range(NCORES):
        rows = slice(c * DHB, (c + 1) * DHB)

        def warr(Wm):
            # rows slice of W, transposed: (E, 128) -> (128 p, 8 c, 128 d)
            return np.ascontiguousarray(
                Wm[rows].T.reshape(8, 128, 128).transpose(1, 0, 2)
            ).astype(NPBF).reshape(128, -1)

        # s_k 2-packed: (128 k, 2 g, 2 h, 64 p, 256 j); k<64 -> d of i1=64g+p,
        # k>=64 -> d of i2=128+64g+p
        sl = s_k[:, rows].reshape(S, S, 2, 64)          # (i, j, h, d)
        vt = sl.transpose(2, 3, 0, 1)                    # (h, d, i, j)
        lo = vt[:, :, 0:128, :].reshape(2, 64, 2, 64, S)   # (h, d, g, p, j)
        hi = vt[:, :, 128:256, :].reshape(2, 64, 2, 64, S)
        skP = np.empty((128, 2, 2, 64, S), NPBF)         # (k, g, h, p, j)
        skP[0:64] = lo.transpose(1, 2, 0, 3, 4)
        skP[64:128] = hi.transpose(1, 2, 0, 3, 4)

        # mask -> (128 rows, 4 gi, 256) with rows in the (g) packed i-order
        mn = np.empty((128, 4, S), np.float32)
        for g in range(2):
            for h in range(2):
                mh = (1.0 - mask[2 * c + h].astype(np.float32)) * SCALING
                gi = 2 * g + h
                mn[0:64, gi] = mh[64 * g:64 * g + 64]
                mn[64:128, gi] = mh[128 + 64 * g:192 + 64 * g]

        maps.append({
            "skT": np.ascontiguousarray(skP).reshape(128, S * S),
            **xs,
            "wq": warr(W["Wq"]), "wk": warr(W["Wk"]), "wr": warr(W["Wr"]),
            "wv": warr(W["Wv"]),
            "wo": np.ascontiguousarray(W["Wo"][:, rows].T).astype(NPBF),
            "sq": np.ascontiguousarray(s_q[0, rows][:, None]),
            "mnot": mn.astype(NPBF).reshape(128, -1),
        })
    return maps


_NC = None


def kernel(**inputs) -> np.ndarray:
    global _NC
    from concourse.bass_utils import run_bass_kernel_spmd

    if _NC is None:
        _NC = build()
    maps = make_in_maps(inputs)
    res = run_bass_kernel_spmd(_NC, maps, list(range(NCORES)))
    out = np.zeros((S, E), np.float32)
    for r in res.results:
        out += np.asarray(r["outp"], np.float32)
    return out


if __name__ == "__main__":
    nc = build()
    print("build ok")
